# revision 9
# baseline (speedup 1.0000x reference)
"""Trainium2 Bass kernel for KeypointSelector:
conv3x3(384->128, pad 1) + bias + ReLU -> conv1x1(128->1) + bias + sigmoid.

Input  dino_features: (32, 64, 64, 384) f32
Output (32, 64, 64, 1) f32

Strategy: pure data parallel over batch, 4 images per core on 8 cores.
The conv3x3 contraction (3 cin chunks of 128 x 9 taps = 27 K-chunks) runs
in fp8e4 DoubleRow mode: each matmul contracts TWO 128-deep K-chunks per
pass (157 TF/s vs 78.6 bf16), so 14 DoubleRow matmuls replace 27 bf16 ones.
K-chunk pairs must sit at an even SBUF column stride; with the padded-row
layout (pitch 66) the pairs are:
  - (ch,dy,dx=-1)+(ch,dy,dx=+1): stride 2 (9 pairs)
  - within dx=0: strides 66/4224 (4 pairs + 1 zero-padded pair)
Inputs are quantized host-side with power-of-two scales (x*32, w*8192 --
exact in fp8/f32), and the 2^-18 descale is folded into the ReLU
activation's scale, so quantization is the only added error (~5e-3 rel).
The 1x1 conv + sigmoid stay in bf16/f32, software-pipelined one tile
behind the conv so the PE never waits on the ACT engine.
"""

import ml_dtypes
import numpy as np

import concourse.bass as bass
import concourse.tile as tile
from concourse import bacc, mybir
from concourse.ap import AP
from concourse.bass_utils import run_bass_kernel_spmd

BF16 = ml_dtypes.bfloat16
FP8 = ml_dtypes.float8_e4m3

# Geometry
B, H, W, CIN, CHID = 32, 64, 64, 384, 128
NCORES = 8
BLOC = B // NCORES  # 4 images per core
HP, WP = H + 2, W + 2  # 66x66 padded grid
NPIX = HP * WP  # 4356 padded pixels per image (one cin chunk)
NCHUNK = CIN // 128  # 3 cin chunks
XT = NCHUNK * NPIX  # 13068 columns: chunks laid out contiguously
TS = 512  # matmul free-dim tile (one PSUM bank of fp32)
START = WP + 1  # padded idx of first valid output pixel (1,1) = 67
END = H * WP + W + 1  # 4289: one past padded idx of pixel (63,63)
NT = -(-(END - START) // TS)  # 9 tiles per image (last one partial)
TILE_N = [min(TS, END - START - t * TS) for t in range(NT)]  # [512]*8 + [126]
OUTW = 64 * WP  # out_s columns actually read by the output DMA (4224)

# fp8 quantization scales (powers of two: scaling is exact)
S_X = 32.0
S_W = 8192.0
DESCALE = 1.0 / (S_X * S_W)  # 2^-18, exact in f32

NPAIR = 14


def _pair_spec():
    """DoubleRow pairing of the 27 K-chunks (ch, dy, dx).

    Returns [(k0, k1, base, delta)]: k0/k1 identify the tap+chunk (k1 None
    = zero-padded partner), base = SBUF column offset of k0 relative to the
    tile's first output pixel, delta = even column stride between k0, k1.
    """
    addr = lambda ch, dy, dx: NPIX * ch + WP * dy + dx
    pairs = []
    for ch in range(NCHUNK):
        for dy in (-1, 0, 1):
            pairs.append(((ch, dy, -1), (ch, dy, 1), addr(ch, dy, -1), 2))
    pairs.append(((0, -1, 0), None, addr(0, -1, 0), 2))
    pairs.append(((0, 0, 0), (0, 1, 0), addr(0, 0, 0), WP))
    pairs.append(((1, -1, 0), (1, 0, 0), addr(1, -1, 0), WP))
    pairs.append(((1, 1, 0), (2, -1, 0), addr(1, 1, 0), NPIX - 2 * WP))
    pairs.append(((2, 0, 0), (2, 1, 0), addr(2, 0, 0), WP))
    assert len(pairs) == NPAIR
    return pairs


_CACHED = {}


def _build_bass(reps=1, reload=True):
    nc = bacc.Bacc("TRN2", target_bir_lowering=False)

    f32 = mybir.dt.float32
    bf16 = mybir.dt.bfloat16
    fp8 = mybir.dt.float8e4

    x = nc.dram_tensor("x", [BLOC, 128, XT], fp8, kind="ExternalInput")
    w1 = nc.dram_tensor("w1", [128, NPAIR, 2, CHID], fp8, kind="ExternalInput")
    b1 = nc.dram_tensor("b1", [CHID, 1], f32, kind="ExternalInput")
    w2 = nc.dram_tensor("w2", [CHID, 1], bf16, kind="ExternalInput")
    b2 = nc.dram_tensor("b2", [1, 1], f32, kind="ExternalInput")
    y = nc.dram_tensor("y", [BLOC, H, W], f32, kind="ExternalOutput")

    pairs = _pair_spec()

    with tile.TileContext(nc) as tc:
        with (
            tc.tile_pool(name="consts", bufs=1) as consts,
            tc.tile_pool(name="xin", bufs=3 if reload else 1) as xin,
            tc.tile_pool(name="hbuf", bufs=3) as hbuf,
            tc.tile_pool(name="obuf", bufs=2) as obuf,
            tc.tile_pool(name="ps1", bufs=3, space="PSUM") as ps1,
            tc.tile_pool(name="ps2", bufs=2, space="PSUM") as ps2,
        ):
            # Constants: conv weights + biases, resident for the whole
            # kernel. w1 gates tile 0, so split it across both HWDGE
            # queues; the tiny biases ride behind.
            w1_s = consts.tile([128, NPAIR, 2, CHID], fp8)
            nc.sync.dma_start(out=w1_s[:, :NPAIR // 2], in_=w1[:, :NPAIR // 2])
            nc.scalar.dma_start(out=w1_s[:, NPAIR // 2:], in_=w1[:, NPAIR // 2:])
            b1_s = consts.tile([CHID, 1], f32)
            nc.scalar.dma_start(out=b1_s, in_=b1[:])
            w2_s = consts.tile([CHID, 1], bf16)
            nc.scalar.dma_start(out=w2_s, in_=w2[:])
            b2_s = consts.tile([1, 1], f32)
            nc.scalar.dma_start(out=b2_s, in_=b2[:])

            # One-tile software pipeline: the 1x1 matmul for tile t-1 is
            # emitted after tile t's conv matmuls, so PE never waits on the
            # ACT ReLU. `pend` carries (h_s, out_s, col, n, last_of_image).
            pend = None

            def flush(pend):
                h_p, out_p, col, n, rows = pend
                p2 = ps2.tile([1, TS], f32)
                nc.tensor.matmul(out=p2[0:1, :n], lhsT=w2_s[:], rhs=h_p[:, :n],
                                 start=True, stop=True)
                nc.scalar.activation(
                    out=out_p[0:1, col:col + n], in_=p2[0:1, :n],
                    func=mybir.ActivationFunctionType.Sigmoid,
                    bias=b2_s[0:1], scale=1.0,
                )
                if rows is not None:
                    # Row range done: write back those valid pixels. Padded
                    # idx of (h,w) is START + 66*h + w -> out_s col 66*h + w.
                    img, r0, r1 = rows
                    src = out_p[0:1, :OUTW].rearrange("p (h w) -> p h w", w=WP)
                    nc.sync.dma_start(out=y[img, r0:r1],
                                      in_=src[:, r0:r1, 0:W])

            preloaded = {}
            if not reload:  # benchmark mode: load all images once up front
                for i in range(BLOC):
                    xc = xin.tile([128, XT], fp8, tag=f"xp{i}")
                    nc.sync.dma_start(out=xc[:], in_=x[i])
                    preloaded[i] = xc

            first = True
            for i in [ii for _ in range(reps) for ii in range(BLOC)]:
                if reload:
                    xc = xin.tile([128, XT], fp8, tag="xs")
                    if first:
                        # Image 0 gates startup: conv tile t reads a
                        # [512t, 512t+645] window from EACH cin chunk, so
                        # stream per-tile block triplets (c0+c2 on the SP
                        # queue, c1 on ACT) in tile order -- tile t
                        # unblocks after triplet t+1, and DMA outruns the
                        # PE from there.
                        segs = []
                        for t in range(NT):
                            a, b = t * TS, min((t + 1) * TS, NPIX)
                            segs.append((a, b, nc.sync))
                            segs.append((2 * NPIX + a, 2 * NPIX + b, nc.sync))
                            segs.append((NPIX + a, NPIX + b, nc.scalar))
                        first = False
                    else:
                        seg = -(-XT // 4)
                        segs = [
                            (g * seg, min((g + 1) * seg, XT), eng)
                            for g, eng in enumerate(
                                [nc.sync, nc.sync, nc.scalar, nc.scalar])
                        ]
                    for a, b, eng in segs:
                        eng.dma_start(out=xc[:, a:b], in_=x[i, :, a:b])
                else:
                    xc = preloaded[i]
                pstride = xc.ap[0][0]

                out_s = obuf.tile([1, OUTW], f32)
                for t in range(NT):
                    n = TILE_N[t]
                    s0 = START + t * TS
                    p1 = ps1.tile([CHID, TS], f32)
                    for p, (k0, k1, base, delta) in enumerate(pairs):
                        rhs = AP(xc.tensor, xc.offset + s0 + base,
                                 [[pstride, 128], [delta, 2], [1, n]])
                        nc.tensor.matmul(
                            out=p1[:, :n],
                            lhsT=w1_s[:, p, :, :],
                            rhs=rhs,
                            start=(p == 0),
                            stop=(p == NPAIR - 1),
                            perf_mode=mybir.MatmulPerfMode.DoubleRow,
                        )
                    if pend is not None:
                        flush(pend)
                    # h = relu(conv/2^18 + b1), rounded to bf16 for the 1x1
                    h_s = hbuf.tile([CHID, TS], bf16)
                    nc.scalar.activation(
                        out=h_s[:, :n], in_=p1[:, :n],
                        func=mybir.ActivationFunctionType.Relu,
                        bias=b1_s[:], scale=DESCALE,
                    )
                    # out_s col 66h+w: rows 0-31 end at col 2109 < 2560, so
                    # they can ship after tile 4's sigmoid; rows 32-63 after
                    # the last tile. Halves the output-DMA tail.
                    rows = None
                    if t == 4:
                        rows = (i, 0, 32)
                    elif t == NT - 1:
                        rows = (i, 32, 64)
                    pend = (h_s, out_s, t * TS, n, rows)
            flush(pend)
    nc.compile()
    return nc


def _prep_inputs(dino_features, W1, b1, W2, b2):
    # x: pad to 66x66, scale by S_X, quantize to fp8e4, lay out as
    # [img, cin_in_chunk(partition), chunk-major padded pixel column]
    xp = np.zeros((B, HP, WP, CIN), dtype=np.float32)
    xp[:, 1:H + 1, 1:W + 1, :] = np.clip(
        np.asarray(dino_features, dtype=np.float32) * S_X, -240.0, 240.0)
    xq = xp.astype(FP8)  # quantize once, then pure relayout
    xt = xq.transpose(0, 3, 1, 2).reshape(B, NCHUNK, 128, NPIX)
    xbuf = np.ascontiguousarray(xt.transpose(0, 2, 1, 3)).reshape(B, 128, XT)

    # W1 (3,3,384,128) (ky,kx,ci,co) -> DoubleRow pairs [128, p, 2, cout]
    w1f = np.asarray(W1, dtype=np.float32) * S_W
    w1h = np.zeros((128, NPAIR, 2, CHID), dtype=FP8)
    for p, (k0, k1, _, _) in enumerate(_pair_spec()):
        for slot, k in ((0, k0), (1, k1)):
            if k is None:
                continue
            ch, dy, dx = k
            w1h[:, p, slot, :] = w1f[dy + 1, dx + 1,
                                     ch * 128:(ch + 1) * 128, :].astype(FP8)

    b1h = np.ascontiguousarray(np.asarray(b1, np.float32).reshape(CHID, 1))
    w2h = np.ascontiguousarray(np.asarray(W2).reshape(CHID, 1).astype(BF16))
    b2h = np.ascontiguousarray(np.asarray(b2, np.float32).reshape(1, 1))

    in_maps = []
    for c in range(NCORES):
        in_maps.append({
            "x": np.ascontiguousarray(xbuf[c * BLOC:(c + 1) * BLOC]),
            "w1": w1h, "b1": b1h, "w2": w2h, "b2": b2h,
        })
    return in_maps


def kernel(dino_features, W1, b1, W2, b2, _trace=False, _trace_kwargs=None):
    if "nc" not in _CACHED:
        _CACHED["nc"] = _build_bass()
    nc = _CACHED["nc"]
    in_maps = _prep_inputs(dino_features, W1, b1, W2, b2)
    res = run_bass_kernel_spmd(nc, in_maps, core_ids=list(range(NCORES)),
                               trace=_trace, **(_trace_kwargs or {}))
    _CACHED["last_results"] = res
    out = np.concatenate([res.results[c]["y"] for c in range(NCORES)], axis=0)
    return out.reshape(B, H, W, 1).astype(np.float32)
